# revision 33
# baseline (speedup 1.0000x reference)
"""Multi-head attention (B=4, S=2048, D=1024, H=16) on 8 TRN2 NeuronCores.

Sharding: core c handles batch b = c % 4 and head-half hh = c // 4
(8 of 16 heads, i.e. 512 of 1024 d_model columns of Wq/Wk/Wv and rows
of Wo).  Each core computes a partial output projection Y_c [S, D];
the host sums the two half partials per batch and adds (bv @ Wo + bo)
(exact because softmax rows sum to 1, so the V-bias contribution
commutes through attention into a constant).

Device layouts (per core), all f32 storage with float32r matmuls:
  x^T [d_model, S] streamed in (host pre-transposes)
  Q^T, K^T  [512, S]  (d-partition)  -> scores^T = K_h Q_h^T directly
  V         [S, 512] with a ones column appended per head ->
     PV matmul accumulates [out^T ; rowsum] in PSUM [65, qchunk]
  softmax is unnormalized exp(s/8); normalization multiplies 1/rowsum
  into the PSUM->SBUF evacuation of out^T (GPSIMD partition broadcast).
"""

import contextlib
import ctypes
import sys
import types

import numpy as np

B, S, D, H = 4, 2048, 1024, 16
DK = D // H  # 64
DH = D // 2  # 512 columns per core (8 heads)
N_CORES = 8

QC = 512  # q chunk (moving dim of scores / PV matmuls)
N_QC = S // QC  # 4
N_KT = S // 128  # 16 k tiles
N_MC = D // 128  # 8 contraction chunks for QKV projections
N_DT = DH // 128  # 4 d-tiles of Q^T/K^T, also chunks of A^T
N_PAIR = 4  # head pairs per core (8 local heads)


def _install_ntff_hook():
    """Provide antenv.axon_hooks if the container lacks it (for trace=True)."""
    if "antenv.axon_hooks" in sys.modules:
        return
    try:
        from antenv.axon_hooks import get_axon_ntff_profile_hook  # noqa: F401

        return
    except ImportError:
        pass
    hook = None
    try:
        lib = ctypes.CDLL("/opt/axon/libaxon_pjrt.so")
        if hasattr(lib, "axon_start_nrt_profile"):
            lib.axon_start_nrt_profile.argtypes = [
                ctypes.POINTER(ctypes.c_int64),
                ctypes.c_size_t,
            ]
            lib.axon_start_nrt_profile.restype = ctypes.c_int64
            lib.axon_stop_nrt_profile.argtypes = [ctypes.c_char_p]
            lib.axon_stop_nrt_profile.restype = ctypes.c_int64

            @contextlib.contextmanager
            def _hook(output_dir, device_ids):
                import jax

                jax.devices()
                if device_ids:
                    ids = (ctypes.c_int64 * len(device_ids))(*device_ids)
                    rc = lib.axon_start_nrt_profile(ids, len(device_ids))
                else:
                    rc = lib.axon_start_nrt_profile(None, 0)
                if rc != 0:
                    raise RuntimeError(f"axon_start_nrt_profile rc={rc}")
                try:
                    yield
                finally:
                    n = lib.axon_stop_nrt_profile(str(output_dir).encode())
                    print(f"profile: {n} file(s) written to {output_dir}")

            hook = _hook
    except OSError:
        pass
    mod = types.ModuleType("antenv.axon_hooks")
    mod.get_axon_ntff_profile_hook = lambda: hook
    mod.set_axon_ntff_profile_hook = lambda h: None
    sys.modules["antenv.axon_hooks"] = mod


def build_nc():
    import concourse.bass as bass
    import concourse.tile as tile
    from concourse import bacc, mybir
    from concourse.bass import ts
    from concourse.tile import add_dep_helper

    f32 = mybir.dt.float32
    f16 = mybir.dt.float16
    EXP = mybir.ActivationFunctionType.Exp

    nc = bacc.Bacc("TRN2", target_bir_lowering=False, debug=False, num_devices=N_CORES)

    xq_d = nc.dram_tensor("xqT", [D, S], f16, kind="ExternalInput")
    xk_d = nc.dram_tensor("xkT", [D, S], f16, kind="ExternalInput")
    xv_d = nc.dram_tensor("xvT", [D, S], f16, kind="ExternalInput")
    wq_d = nc.dram_tensor("wq", [D, DH], f16, kind="ExternalInput")
    wk_d = nc.dram_tensor("wk", [D, DH], f16, kind="ExternalInput")
    wv_d = nc.dram_tensor("wv", [D, DH], f16, kind="ExternalInput")
    wo_d = nc.dram_tensor("wo", [DH, D], f16, kind="ExternalInput")
    bq_d = nc.dram_tensor("bq", [128, N_DT], f32, kind="ExternalInput")
    bk_d = nc.dram_tensor("bk", [128, N_DT], f32, kind="ExternalInput")
    y_d = nc.dram_tensor("y", [S, D], f32, kind="ExternalOutput")

    with tile.TileContext(nc) as tc:
        with contextlib.ExitStack() as ctx:
            consts = ctx.enter_context(tc.tile_pool(name="consts", bufs=1))
            xpool = ctx.enter_context(tc.tile_pool(name="xpool", bufs=1))
            wpool = ctx.enter_context(tc.tile_pool(name="wpool", bufs=1))
            work = ctx.enter_context(tc.tile_pool(name="work", bufs=3))
            norm = ctx.enter_context(tc.tile_pool(name="norm", bufs=2))
            psum = ctx.enter_context(tc.tile_pool(name="psum", bufs=1, space="PSUM"))

            bq_sb = consts.tile([128, N_DT], f32, tag="bq")
            bk_sb = consts.tile([128, N_DT], f32, tag="bk")
            nc.sync.dma_start(out=bq_sb, in_=bq_d[:, :])
            nc.sync.dma_start(out=bk_sb, in_=bk_d[:, :])

            # persistent activations
            qt_sb = consts.tile([128, N_DT, S], f16, tag="qt")  # Q^T [d, s]
            kt_sb = consts.tile([128, N_DT, S], f16, tag="kt")  # K^T [d, s]
            # V with per-head ones column: [s-in-tile, s-tile, head, 65]
            vp_sb = consts.tile([128, N_KT, 8, DK + 1], f16, tag="vp")

            nc.vector.memset(vp_sb[:, :, :, DK : DK + 1], 1.0)

            # ---- QKV projections ------------------------------------------
            def load_x(x_d):
                # split by contraction-chunk pairs so the first matmuls can
                # start as soon as the leading chunks arrive (subtile deps)
                x_sb = xpool.tile([128, N_MC, S], f16, tag="x", name="x", bufs=2)
                xr = x_d.rearrange("(j p) s -> p j s", p=128)
                for jj in range(0, N_MC, 2):
                    nc.sync.dma_start(
                        out=x_sb[:, jj : jj + 2, :], in_=xr[:, jj : jj + 2, :]
                    )
                return x_sb

            def load_w(w_d):
                w_sb = wpool.tile([128, N_MC, DH], f16, tag="w", name="w", bufs=2)
                nc.sync.dma_start(
                    out=w_sb, in_=w_d.rearrange("(j p) d -> p j d", p=128)
                )
                return w_sb

            def proj_T(x_sb, w_sb, b_sb, out_sb):
                # out^T[d, s] = sum_m W[m, d] * x^T[m, s]   (+ bias[d]);
                # two q-chunks share one 2-bank psum tile -> one wide evac
                for t in range(N_DT):
                    for qc in range(0, N_QC, 2):
                        ps = psum.tile(
                            [128, 2, QC], f32, tag="ps_s", bufs=2, name="ps"
                        )
                        for j in range(N_MC):
                            for u in range(2):
                                nc.tensor.matmul(
                                    ps[:, u, :],
                                    lhsT=(w_sb[:, j, ts(t, 128)]),
                                    rhs=(x_sb[:, j, ts(qc + u, QC)]),
                                    start=(j == 0),
                                    stop=(j == N_MC - 1),
                                )
                        nc.vector.tensor_scalar_add(
                            out_sb[:, t, qc * QC : (qc + 2) * QC],
                            ps.rearrange("p u q -> p (u q)"),
                            b_sb[:, t : t + 1],
                        )

            def proj_V(x_sb, w_sb):
                # V[s, d] = sum_m x^T[m, s] * W[m, d]; no bias (folded on host)
                for st in range(0, N_KT, 2):
                    ps = psum.tile([128, 2, DH], f32, tag="ps_s", bufs=2, name="ps")
                    for j in range(N_MC):
                        for u in range(2):
                            nc.tensor.matmul(
                                ps[:, u, :],
                                lhsT=(x_sb[:, j, ts(st + u, 128)]),
                                rhs=(w_sb[:, j, :]),
                                start=(j == 0),
                                stop=(j == N_MC - 1),
                            )
                    nc.scalar.copy(
                        vp_sb[:, st : st + 2, :, 0:DK],
                        ps.rearrange("p u (l d) -> p u l d", l=8),
                    )

            with nc.named_scope("proj_v"):
                xv_sb = load_x(xv_d)
                wv_sb = load_w(wv_d)
                proj_V(xv_sb, wv_sb)
            with nc.named_scope("proj_k"):
                xk_sb = load_x(xk_d)
                wk_sb = load_w(wk_d)
                proj_T(xk_sb, wk_sb, bk_sb, kt_sb)
            with nc.named_scope("proj_q"):
                xq_sb = load_x(xq_d)
                wq_sb = load_w(wq_d)
                proj_T(xq_sb, wq_sb, bq_sb, qt_sb)

            # A^T reuses the (released) x slot — same pool tag, bufs=1
            at_sb = xpool.tile([128, N_DT, S], f16, tag="x", name="at", bufs=2)

            # output projection weights, loaded after the x/w traffic
            wo_sb = consts.tile([128, N_DT, D], f16, tag="wo")
            nc.sync.dma_start(out=wo_sb, in_=wo_d.rearrange("(c p) o -> p c o", p=128))

            # ---- attention + output projection, per q chunk ---------------
            for qc in range(N_QC):
                with nc.named_scope(f"attn_qc{qc}"):
                    for hp in range(N_PAIR):
                        t = hp  # d-tile holding this head pair
                        po = []
                        for half in range(2):
                            po.append(
                                psum.tile(
                                    [DK + 1, QC], f32, tag="ps_o", bufs=3, name="ps_o"
                                )
                            )
                        # software pipeline: scores(kt) pair issued adjacently
                        # (concurrent row groups), PV lags one step so it never
                        # waits on the exp of the same kt.  PE program order is
                        # pinned to s1,s2,pv1,pv2 per kt so each LDWEIGHTS can
                        # overlap a matmul on the other row group.
                        prev_pe = None

                        def chain(mm):
                            nonlocal prev_pe
                            if prev_pe is not None:
                                add_dep_helper(
                                    mm.ins, prev_pe, sync=False, reason="pe-order"
                                )
                            prev_pe = mm.ins

                        pu_prev = None
                        for kt in range(N_KT + 1):
                            pu_cur = None
                            if kt < N_KT:
                                # both heads' scores into one 2-bank PSUM tile
                                # so a single [128,1024] exp covers the pair
                                ps_s = psum.tile(
                                    [128, 2, QC], f32, tag="ps_s", bufs=2, name="ps_s"
                                )
                                for half in range(2):
                                    hb = half * DK
                                    chain(
                                        nc.tensor.matmul(
                                            ps_s[:, half, :],
                                            lhsT=(kt_sb[hb : hb + DK, t, ts(kt, 128)]),
                                            rhs=(qt_sb[hb : hb + DK, t, ts(qc, QC)]),
                                            start=True,
                                            stop=True,
                                        )
                                    )
                            if pu_prev is not None:
                                for half in range(2):
                                    chain(
                                        nc.tensor.matmul(
                                            po[half],
                                            lhsT=(vp_sb[:, kt - 1, 2 * hp + half, :]),
                                            rhs=pu_prev[:, half, :],
                                            start=(kt == 1),
                                            stop=(kt == N_KT),
                                        )
                                    )
                            if kt < N_KT:
                                pu_cur = work.tile(
                                    [128, 2, QC], f16, tag="pu", bufs=4, name="pu"
                                )
                                nc.scalar.activation(
                                    pu_cur, ps_s[:], EXP, scale=0.125
                                )
                            pu_prev = pu_cur
                        for half in range(2):
                            hb = half * DK
                            srow = norm.tile([1, QC], f32, tag="srow")
                            nc.vector.tensor_copy(out=srow, in_=po[half][DK : DK + 1, :])
                            recip = norm.tile([1, QC], f32, tag="recip")
                            nc.vector.reciprocal_approx_fast(out=recip, in_=srow[:])
                            bc = norm.tile([DK, QC], f32, tag="bc")
                            nc.gpsimd.partition_broadcast(bc, recip[:])
                            nc.vector.tensor_mul(
                                at_sb[hb : hb + DK, t, ts(qc, QC)],
                                po[half][0:DK, :],
                                bc,
                            )
                with nc.named_scope(f"oproj_qc{qc}"):
                    # Y[s, :] for the 4 q-subtiles of this chunk
                    for sq in range(QC // 128):
                        qt = qc * (QC // 128) + sq
                        for oc in range(D // 512):
                            ps_y = psum.tile(
                                [128, 512], f32, tag="ps_y", bufs=1, name="ps_y"
                            )
                            for ch in range(N_DT):
                                nc.tensor.matmul(
                                    ps_y,
                                    lhsT=(at_sb[:, ch, ts(qt, 128)]),
                                    rhs=(wo_sb[:, ch, ts(oc, 512)]),
                                    start=(ch == 0),
                                    stop=(ch == N_DT - 1),
                                )
                            y_sb = work.tile([128, 512], f32, tag="y_sb", bufs=2)
                            nc.vector.tensor_copy(out=y_sb, in_=ps_y)
                            nc.sync.dma_start(
                                out=y_d[ts(qt, 128), ts(oc, 512)], in_=y_sb
                            )

    nc.compile()
    return nc


_NC_CACHE = None


def _get_nc():
    global _NC_CACHE
    if _NC_CACHE is None:
        _install_ntff_hook()
        _NC_CACHE = build_nc()
    return _NC_CACHE


def make_in_maps(query, key, value, Wq, bq, Wk, bk, Wv, bv, Wo, bo):
    f = np.float32
    in_maps = []
    for c in range(N_CORES):
        b, hh = c % B, c // B
        cs = slice(hh * DH, (hh + 1) * DH)
        in_maps.append(
            {
                "xqT": np.asarray(query[b], f).T.astype(np.float16),
                "xkT": np.asarray(key[b], f).T.astype(np.float16),
                "xvT": np.asarray(value[b], f).T.astype(np.float16),
                "wq": np.asarray(Wq, f)[:, cs].astype(np.float16),
                "wk": np.asarray(Wk, f)[:, cs].astype(np.float16),
                "wv": np.asarray(Wv, f)[:, cs].astype(np.float16),
                "wo": np.asarray(Wo, f)[cs, :].astype(np.float16),
                "bq": np.ascontiguousarray(np.asarray(bq, f)[cs].reshape(N_DT, 128).T),
                "bk": np.ascontiguousarray(np.asarray(bk, f)[cs].reshape(N_DT, 128).T),
            }
        )
    return in_maps


def postprocess(results, Wo, bv, bo):
    const = np.asarray(bv, np.float32) @ np.asarray(Wo, np.float32) + np.asarray(
        bo, np.float32
    )
    out = np.empty((B, S, D), np.float32)
    for b in range(B):
        out[b] = results[b]["y"] + results[b + B]["y"] + const
    return out


def kernel(query, key, value, Wq, bq, Wk, bk, Wv, bv, Wo, bo, trace=False):
    from concourse.bass_utils import run_bass_kernel_spmd

    nc = _get_nc()
    in_maps = make_in_maps(query, key, value, Wq, bq, Wk, bk, Wv, bv, Wo, bo)
    res = run_bass_kernel_spmd(nc, in_maps, core_ids=list(range(N_CORES)), trace=trace)
    out = postprocess(res.results, Wo, bv, bo)
    if trace:
        kernel.last_result = res
    return out


# revision 34
# speedup vs baseline: 1.0768x; 1.0768x over previous
"""Multi-head attention (B=4, S=2048, D=1024, H=16) on 8 TRN2 NeuronCores.

Sharding: core c handles batch b = c % 4 and head-half hh = c // 4
(8 of 16 heads, i.e. 512 of 1024 d_model columns of Wq/Wk/Wv and rows
of Wo).  Each core computes a partial output projection Y_c [S, D];
the host sums the two half partials per batch and adds (bv @ Wo + bo)
(exact because softmax rows sum to 1, so the V-bias contribution
commutes through attention into a constant).

Device layouts (per core), all f32 storage with float32r matmuls:
  x^T [d_model, S] streamed in (host pre-transposes)
  Q^T, K^T  [512, S]  (d-partition)  -> scores^T = K_h Q_h^T directly
  V         [S, 512] with a ones column appended per head ->
     PV matmul accumulates [out^T ; rowsum] in PSUM [65, qchunk]
  softmax is unnormalized exp(s/8); normalization multiplies 1/rowsum
  into the PSUM->SBUF evacuation of out^T (GPSIMD partition broadcast).
"""

import contextlib
import ctypes
import sys
import types

import numpy as np

B, S, D, H = 4, 2048, 1024, 16
DK = D // H  # 64
DH = D // 2  # 512 columns per core (8 heads)
N_CORES = 8

QC = 512  # q chunk (moving dim of scores / PV matmuls)
N_QC = S // QC  # 4
N_KT = S // 128  # 16 k tiles
N_MC = D // 128  # 8 contraction chunks for QKV projections
N_DT = DH // 128  # 4 d-tiles of Q^T/K^T, also chunks of A^T
N_PAIR = 4  # head pairs per core (8 local heads)


def _install_ntff_hook():
    """Provide antenv.axon_hooks if the container lacks it (for trace=True)."""
    if "antenv.axon_hooks" in sys.modules:
        return
    try:
        from antenv.axon_hooks import get_axon_ntff_profile_hook  # noqa: F401

        return
    except ImportError:
        pass
    hook = None
    try:
        lib = ctypes.CDLL("/opt/axon/libaxon_pjrt.so")
        if hasattr(lib, "axon_start_nrt_profile"):
            lib.axon_start_nrt_profile.argtypes = [
                ctypes.POINTER(ctypes.c_int64),
                ctypes.c_size_t,
            ]
            lib.axon_start_nrt_profile.restype = ctypes.c_int64
            lib.axon_stop_nrt_profile.argtypes = [ctypes.c_char_p]
            lib.axon_stop_nrt_profile.restype = ctypes.c_int64

            @contextlib.contextmanager
            def _hook(output_dir, device_ids):
                import jax

                jax.devices()
                if device_ids:
                    ids = (ctypes.c_int64 * len(device_ids))(*device_ids)
                    rc = lib.axon_start_nrt_profile(ids, len(device_ids))
                else:
                    rc = lib.axon_start_nrt_profile(None, 0)
                if rc != 0:
                    raise RuntimeError(f"axon_start_nrt_profile rc={rc}")
                try:
                    yield
                finally:
                    n = lib.axon_stop_nrt_profile(str(output_dir).encode())
                    print(f"profile: {n} file(s) written to {output_dir}")

            hook = _hook
    except OSError:
        pass
    mod = types.ModuleType("antenv.axon_hooks")
    mod.get_axon_ntff_profile_hook = lambda: hook
    mod.set_axon_ntff_profile_hook = lambda h: None
    sys.modules["antenv.axon_hooks"] = mod


def build_nc():
    import concourse.bass as bass
    import concourse.tile as tile
    from concourse import bacc, mybir
    from concourse.bass import ts
    from concourse.tile import add_dep_helper

    f32 = mybir.dt.float32
    f16 = mybir.dt.float16
    EXP = mybir.ActivationFunctionType.Exp

    nc = bacc.Bacc("TRN2", target_bir_lowering=False, debug=False, num_devices=N_CORES)

    xq_d = nc.dram_tensor("xqT", [D, S], f16, kind="ExternalInput")
    xk_d = nc.dram_tensor("xkT", [D, S], f16, kind="ExternalInput")
    xv_d = nc.dram_tensor("xvT", [D, S], f16, kind="ExternalInput")
    wq_d = nc.dram_tensor("wq", [D, DH], f16, kind="ExternalInput")
    wk_d = nc.dram_tensor("wk", [D, DH], f16, kind="ExternalInput")
    wv_d = nc.dram_tensor("wv", [D, DH], f16, kind="ExternalInput")
    wo_d = nc.dram_tensor("wo", [DH, D], f16, kind="ExternalInput")
    bq_d = nc.dram_tensor("bq", [128, N_DT], f32, kind="ExternalInput")
    bk_d = nc.dram_tensor("bk", [128, N_DT], f32, kind="ExternalInput")
    y_d = nc.dram_tensor("y", [S, D], f32, kind="ExternalOutput")

    with tile.TileContext(nc) as tc:
        with contextlib.ExitStack() as ctx:
            consts = ctx.enter_context(tc.tile_pool(name="consts", bufs=1))
            xpool = ctx.enter_context(tc.tile_pool(name="xpool", bufs=1))
            wpool = ctx.enter_context(tc.tile_pool(name="wpool", bufs=1))
            work = ctx.enter_context(tc.tile_pool(name="work", bufs=3))
            norm = ctx.enter_context(tc.tile_pool(name="norm", bufs=2))
            psum = ctx.enter_context(tc.tile_pool(name="psum", bufs=1, space="PSUM"))

            bq_sb = consts.tile([128, N_DT], f32, tag="bq")
            bk_sb = consts.tile([128, N_DT], f32, tag="bk")
            nc.sync.dma_start(out=bq_sb, in_=bq_d[:, :])
            nc.sync.dma_start(out=bk_sb, in_=bk_d[:, :])

            # persistent activations
            qt_sb = consts.tile([128, N_DT, S], f16, tag="qt")  # Q^T [d, s]
            kt_sb = consts.tile([128, N_DT, S], f16, tag="kt")  # K^T [d, s]
            # V with per-head ones column: [s-in-tile, s-tile, head, 65]
            vp_sb = consts.tile([128, N_KT, 8, DK + 1], f16, tag="vp")

            nc.vector.memset(vp_sb[:, :, :, DK : DK + 1], 1.0)

            # ---- QKV projections ------------------------------------------
            def load_x(x_d):
                # split by contraction-chunk pairs so the first matmuls can
                # start as soon as the leading chunks arrive (subtile deps)
                x_sb = xpool.tile([128, N_MC, S], f16, tag="x", name="x", bufs=2)
                xr = x_d.rearrange("(j p) s -> p j s", p=128)
                for jj in range(0, N_MC, 2):
                    nc.sync.dma_start(
                        out=x_sb[:, jj : jj + 2, :], in_=xr[:, jj : jj + 2, :]
                    )
                return x_sb

            def load_w(w_d):
                w_sb = wpool.tile([128, N_MC, DH], f16, tag="w", name="w", bufs=2)
                nc.sync.dma_start(
                    out=w_sb, in_=w_d.rearrange("(j p) d -> p j d", p=128)
                )
                return w_sb

            def proj_T(x_sb, w_sb, b_sb, out_sb):
                # out^T[d, s] = sum_m W[m, d] * x^T[m, s]   (+ bias[d]);
                # two q-chunks share one 2-bank psum tile -> one wide evac
                for t in range(N_DT):
                    for qc in range(0, N_QC, 2):
                        ps = psum.tile(
                            [128, 2, QC], f32, tag="ps_s", bufs=2, name="ps"
                        )
                        for j in range(N_MC):
                            for u in range(2):
                                nc.tensor.matmul(
                                    ps[:, u, :],
                                    lhsT=(w_sb[:, j, ts(t, 128)]),
                                    rhs=(x_sb[:, j, ts(qc + u, QC)]),
                                    start=(j == 0),
                                    stop=(j == N_MC - 1),
                                )
                        nc.vector.tensor_scalar_add(
                            out_sb[:, t, qc * QC : (qc + 2) * QC],
                            ps.rearrange("p u q -> p (u q)"),
                            b_sb[:, t : t + 1],
                        )

            def proj_V(x_sb, w_sb):
                # V[s, d] = sum_m x^T[m, s] * W[m, d]; no bias (folded on host)
                for st in range(0, N_KT, 2):
                    ps = psum.tile([128, 2, DH], f32, tag="ps_s", bufs=2, name="ps")
                    for j in range(N_MC):
                        for u in range(2):
                            nc.tensor.matmul(
                                ps[:, u, :],
                                lhsT=(x_sb[:, j, ts(st + u, 128)]),
                                rhs=(w_sb[:, j, :]),
                                start=(j == 0),
                                stop=(j == N_MC - 1),
                            )
                    nc.scalar.copy(
                        vp_sb[:, st : st + 2, :, 0:DK],
                        ps.rearrange("p u (l d) -> p u l d", l=8),
                    )

            with nc.named_scope("proj_v"):
                xv_sb = load_x(xv_d)
                wv_sb = load_w(wv_d)
                proj_V(xv_sb, wv_sb)
            with nc.named_scope("proj_k"):
                xk_sb = load_x(xk_d)
                wk_sb = load_w(wk_d)
                proj_T(xk_sb, wk_sb, bk_sb, kt_sb)
            with nc.named_scope("proj_q"):
                xq_sb = load_x(xq_d)
                wq_sb = load_w(wq_d)
                proj_T(xq_sb, wq_sb, bq_sb, qt_sb)

            # A^T reuses the (released) x slot — same pool tag, bufs=1
            at_sb = xpool.tile([128, N_DT, S], f16, tag="x", name="at", bufs=2)

            # output projection weights, loaded after the x/w traffic
            wo_sb = consts.tile([128, N_DT, D], f16, tag="wo")
            nc.sync.dma_start(out=wo_sb, in_=wo_d.rearrange("(c p) o -> p c o", p=128))

            # ---- attention + output projection, per q chunk ---------------
            for qc in range(N_QC):
                with nc.named_scope(f"attn_qc{qc}"):
                    for hp in range(N_PAIR):
                        t = hp  # d-tile holding this head pair
                        po = []
                        for half in range(2):
                            po.append(
                                psum.tile(
                                    [DK + 1, QC], f32, tag="ps_o", bufs=4, name="ps_o"
                                )
                            )
                        # software pipeline: scores(kt) pair issued adjacently
                        # (concurrent row groups), PV lags one step so it never
                        # waits on the exp of the same kt.  PE program order is
                        # pinned to s1,s2,pv1,pv2 per kt so each LDWEIGHTS can
                        # overlap a matmul on the other row group.
                        prev_pe = None

                        def chain(mm):
                            nonlocal prev_pe
                            if prev_pe is not None:
                                add_dep_helper(
                                    mm.ins, prev_pe, sync=False, reason="pe-order"
                                )
                            prev_pe = mm.ins

                        pu_prev = None
                        for kt in range(N_KT + 1):
                            pu_cur = None
                            if kt < N_KT:
                                # both heads' scores into one 2-bank PSUM tile
                                # so a single [128,1024] exp covers the pair
                                ps_s = psum.tile(
                                    [128, 2, QC], f32, tag="ps_s", bufs=2, name="ps_s"
                                )
                                for half in range(2):
                                    hb = half * DK
                                    chain(
                                        nc.tensor.matmul(
                                            ps_s[:, half, :],
                                            lhsT=(kt_sb[hb : hb + DK, t, ts(kt, 128)]),
                                            rhs=(qt_sb[hb : hb + DK, t, ts(qc, QC)]),
                                            start=True,
                                            stop=True,
                                        )
                                    )
                            if pu_prev is not None:
                                for half in range(2):
                                    chain(
                                        nc.tensor.matmul(
                                            po[half],
                                            lhsT=(vp_sb[:, kt - 1, 2 * hp + half, :]),
                                            rhs=pu_prev[:, half, :],
                                            start=(kt == 1),
                                            stop=(kt == N_KT),
                                        )
                                    )
                            if kt < N_KT:
                                pu_cur = work.tile(
                                    [128, 2, QC], f16, tag="pu", bufs=4, name="pu"
                                )
                                nc.scalar.activation(
                                    pu_cur, ps_s[:], EXP, scale=0.125
                                )
                            pu_prev = pu_cur
                        for half in range(2):
                            hb = half * DK
                            srow = norm.tile([1, QC], f32, tag="srow")
                            nc.vector.tensor_copy(out=srow, in_=po[half][DK : DK + 1, :])
                            recip = norm.tile([1, QC], f32, tag="recip")
                            nc.vector.reciprocal_approx_fast(out=recip, in_=srow[:])
                            bc = norm.tile([DK, QC], f32, tag="bc")
                            nc.gpsimd.partition_broadcast(bc, recip[:])
                            nc.vector.tensor_mul(
                                at_sb[hb : hb + DK, t, ts(qc, QC)],
                                po[half][0:DK, :],
                                bc,
                            )
                with nc.named_scope(f"oproj_qc{qc}"):
                    # Y[s, :] for the 4 q-subtiles of this chunk
                    for sq in range(QC // 128):
                        qt = qc * (QC // 128) + sq
                        for oc in range(D // 512):
                            ps_y = psum.tile(
                                [128, 512], f32, tag="ps_s", bufs=2, name="ps_y"
                            )
                            for ch in range(N_DT):
                                nc.tensor.matmul(
                                    ps_y,
                                    lhsT=(at_sb[:, ch, ts(qt, 128)]),
                                    rhs=(wo_sb[:, ch, ts(oc, 512)]),
                                    start=(ch == 0),
                                    stop=(ch == N_DT - 1),
                                )
                            y_sb = work.tile([128, 512], f32, tag="y_sb", bufs=2)
                            nc.vector.tensor_copy(out=y_sb, in_=ps_y)
                            nc.sync.dma_start(
                                out=y_d[ts(qt, 128), ts(oc, 512)], in_=y_sb
                            )

    nc.compile()
    return nc


_NC_CACHE = None


def _get_nc():
    global _NC_CACHE
    if _NC_CACHE is None:
        _install_ntff_hook()
        _NC_CACHE = build_nc()
    return _NC_CACHE


def make_in_maps(query, key, value, Wq, bq, Wk, bk, Wv, bv, Wo, bo):
    f = np.float32
    in_maps = []
    for c in range(N_CORES):
        b, hh = c % B, c // B
        cs = slice(hh * DH, (hh + 1) * DH)
        in_maps.append(
            {
                "xqT": np.asarray(query[b], f).T.astype(np.float16),
                "xkT": np.asarray(key[b], f).T.astype(np.float16),
                "xvT": np.asarray(value[b], f).T.astype(np.float16),
                "wq": np.asarray(Wq, f)[:, cs].astype(np.float16),
                "wk": np.asarray(Wk, f)[:, cs].astype(np.float16),
                "wv": np.asarray(Wv, f)[:, cs].astype(np.float16),
                "wo": np.asarray(Wo, f)[cs, :].astype(np.float16),
                "bq": np.ascontiguousarray(np.asarray(bq, f)[cs].reshape(N_DT, 128).T),
                "bk": np.ascontiguousarray(np.asarray(bk, f)[cs].reshape(N_DT, 128).T),
            }
        )
    return in_maps


def postprocess(results, Wo, bv, bo):
    const = np.asarray(bv, np.float32) @ np.asarray(Wo, np.float32) + np.asarray(
        bo, np.float32
    )
    out = np.empty((B, S, D), np.float32)
    for b in range(B):
        out[b] = results[b]["y"] + results[b + B]["y"] + const
    return out


def kernel(query, key, value, Wq, bq, Wk, bk, Wv, bv, Wo, bo, trace=False):
    from concourse.bass_utils import run_bass_kernel_spmd

    nc = _get_nc()
    in_maps = make_in_maps(query, key, value, Wq, bq, Wk, bk, Wv, bv, Wo, bo)
    res = run_bass_kernel_spmd(nc, in_maps, core_ids=list(range(N_CORES)), trace=trace)
    out = postprocess(res.results, Wo, bv, bo)
    if trace:
        kernel.last_result = res
    return out
